# revision 1
# baseline (speedup 1.0000x reference)
"""MixLinear int4-GEMM kernel for 8x TRN2 NeuronCores.

Strategy: tensor-parallel over out_features (each core owns OUT/8 = 512
output channels; q_weight / scale_col / weight_cache are sharded along the
output dim; x is replicated).  Per core:

  1. Per 128-row activation tile: masked abs-max (outlier cols excluded) in
     one fused DVE tensor_tensor_reduce pass; x_scale = max/7, r = 1/x_scale.
  2. Magic-number RNE round: t = xz*r + 1.5*2^23 on ScalarE, q = t - magic on
     GPSIMD (exact small ints, cast to bf16).
  3. q is transposed to contraction-major layout with one DMA-xbar transpose.
  4. int4 weights are unpacked on-device (DVE bitwise ops on the packed
     bytes) into a [K, 32, 512] bf16 wT resident in SBUF, transposed by
     DMA-xbar.
  5. 32 bf16 matmuls (exact: integer values) + 2 outlier matmuls accumulate
     into one PSUM bank.  The outlier operands are pre-scaled by 1/x_scale
     (per row) and 1/scale_col (per out channel) so one dequant applies to
     the whole PSUM: y = psum * x_scale * scale_col, fused into the PSUM
     eviction (ScalarE per-partition scale, DVE broadcast multiply).

The output shard [8192, 512] is DMA'd out; the host concatenates shards.
"""

import numpy as np

B, S, IN, OUT, FP = 4, 2048, 4096, 4096, 256
M = B * S
NCORES = 8
OS = OUT // NCORES  # out-features shard per core
QMAX = 7.0
MAGIC = 12582912.0  # 1.5 * 2**23: adding+subtracting forces RNE to integer


def emit_core_kernel(nc, tc, m, in_dim, os_dim, fp_dim):
    """Emit the per-core tile program. All dims compile-time constants."""
    import os as _os

    import concourse.bass as bass
    import concourse.mybir as mybir
    from concourse.masks import make_identity

    DBG = set(_os.environ.get("KERNEL_DISABLE", "").split(","))

    f32 = mybir.dt.float32
    f32r = mybir.dt.float32r
    bf16 = mybir.dt.bfloat16
    i32 = mybir.dt.int32
    i16 = mybir.dt.int16
    Alu = mybir.AluOpType
    Act = mybir.ActivationFunctionType

    P = 128
    MT = m // P              # number of 128-row activation tiles
    KT = in_dim // P         # number of 128-deep contraction tiles
    FT = fp_dim // P         # outlier contraction tiles (2)
    OJ = os_dim // P         # out-shard subtiles (4)

    x = nc.dram_tensor("x", [m, in_dim], f32, kind="ExternalInput")
    qw = nc.dram_tensor("qw", [os_dim, in_dim // 2], i32, kind="ExternalInput")
    sc = nc.dram_tensor("sc", [os_dim], f32, kind="ExternalInput")
    wc = nc.dram_tensor("wc", [os_dim, fp_dim], f32, kind="ExternalInput")
    maskrow = nc.dram_tensor("maskrow", [in_dim], f32, kind="ExternalInput")
    idx = nc.dram_tensor("idx", [P, fp_dim // 16], i16, kind="ExternalInput")
    y = nc.dram_tensor("y", [m, os_dim], f32, kind="ExternalOutput")

    with (
        tc.tile_pool(name="const", bufs=1) as const,
        tc.tile_pool(name="wstage", bufs=1) as wstage,
        tc.tile_pool(name="xp", bufs=2) as xp,
        tc.tile_pool(name="xzp", bufs=2) as xzp,
        tc.tile_pool(name="qp", bufs=2) as qp,
        tc.tile_pool(name="qtp", bufs=2) as qtp,
        tc.tile_pool(name="aop", bufs=2) as aop,
        tc.tile_pool(name="aotp", bufs=2) as aotp,
        tc.tile_pool(name="sp", bufs=6) as sp,
        tc.tile_pool(name="yp", bufs=2) as yp,
        tc.tile_pool(name="py", bufs=2, space="PSUM") as py,
        tc.tile_pool(name="ptp", bufs=2, space="PSUM") as ptp,
    ):
        # ---------------- one-time setup ----------------
        from concourse import library_config

        if "gather" not in DBG:
            nc.gpsimd.load_library(library_config.ap_gather)

        identity = const.tile([P, P], f32)
        make_identity(nc, identity[:])

        # outlier mask broadcast to all partitions: maskF[p, k] = 0 iff k in ind
        maskF = const.tile([P, in_dim], f32)
        nc.sync.dma_start(maskF[:], maskrow[None, :].to_broadcast((P, in_dim)))

        # wrapped gather indices for ap_gather
        idxs = const.tile([P, fp_dim // 16], i16)
        nc.sync.dma_start(idxs[:], idx[:])

        # scale_col shard: broadcast along partitions [P, OS] for dequant
        sc_bcast = const.tile([P, os_dim], f32)
        nc.sync.dma_start(sc_bcast[:], sc[None, :].to_broadcast((P, os_dim)))

        # scale_col per-partition view [P, OJ] for pre-dividing weight_cache
        sc_op = const.tile([P, OJ], f32)
        nc.sync.dma_start(sc_op[:], sc.rearrange("(j p) -> p j", p=P))

        # weight_cache': wc[o, f] / sc[o], transposed to [P_f, FT, OS] bf16
        wc_sb = wstage.tile([P, OJ, fp_dim], f32)
        nc.sync.dma_start(wc_sb[:], wc.rearrange("(j p) f -> p j f", p=P))
        rsc_op = const.tile([P, OJ], f32)
        nc.vector.reciprocal(rsc_op[:], sc_op[:])
        wcp = wstage.tile([P, OJ, fp_dim], f32)
        for j in range(OJ):
            nc.vector.tensor_scalar(
                wcp[:, j, :], wc_sb[:, j, :], rsc_op[:, j : j + 1], None, Alu.mult
            )
        wcT = const.tile([P, FT, os_dim], f32r)
        for j in range(OJ):
            for ff in range(FT):
                ps = ptp.tile([P, P], f32, tag="tp")
                nc.tensor.transpose(ps[:], wcp[:, j, ff * P : (ff + 1) * P], identity[:])
                nc.scalar.activation(
                    wcT[:, ff, j * P : (j + 1) * P], ps[:], Act.Copy
                )

        # int4 weight unpack: qw[o, i] byte -> w[o, 2i] = lo nibble signed,
        # w[o, 2i+1] = hi nibble signed; then DMA-xbar into wT [P_k, KT, OS]
        wT = const.tile([P, KT, os_dim], bf16)
        qw_v = qw.rearrange("(j p) k -> p j k", p=P)
        for j in range(OJ):
            qwj = wstage.tile([P, in_dim // 2], i32, tag="qwj")
            nc.sync.dma_start(qwj[:], qw_v[:, j, :])
            w_ok = wstage.tile([P, in_dim], bf16, tag="wok")
            w_ok_v = w_ok.rearrange("p (k two) -> p k two", two=2)
            tmp = wstage.tile([P, in_dim // 2], i32, tag="wtmp")
            # low nibble: ((v & 15) ^ 8) - 8
            nc.vector.tensor_scalar(
                tmp[:], qwj[:], 15, 8, Alu.bitwise_and, Alu.bitwise_xor
            )
            nc.vector.tensor_scalar(w_ok_v[:, :, 0], tmp[:], 8, None, Alu.subtract)
            # high nibble: (((v >> 4) & 15) ^ 8) - 8
            tmp2 = wstage.tile([P, in_dim // 2], i32, tag="wtmp2")
            nc.vector.tensor_scalar(
                tmp2[:], qwj[:], 4, None, Alu.arith_shift_right
            )
            nc.vector.tensor_scalar(
                tmp[:], tmp2[:], 15, 8, Alu.bitwise_and, Alu.bitwise_xor
            )
            nc.vector.tensor_scalar(w_ok_v[:, :, 1], tmp[:], 8, None, Alu.subtract)
            # transpose [128 o, in_dim k] -> wT[p_k, KT, o-chunk j]
            nc.sync.dma_start_transpose(wT[:, :, j * P : (j + 1) * P], w_ok[:])

        # ---------------- main loop over 128-row activation tiles ----------
        for mi in range(MT):
            x_t = xp.tile([P, in_dim], f32)
            nc.sync.dma_start(x_t[:], x[mi * P : (mi + 1) * P, :])

            # masked abs-max -> mx; xz = x * mask (outlier cols zeroed)
            xz = xzp.tile([P, in_dim], f32)
            mx = sp.tile([P, 1], f32, tag="mx")
            nc.vector.tensor_tensor(xz[:], x_t[:], maskF[:], Alu.mult)
            nc.vector.tensor_reduce(
                mx[:], xz[:], mybir.AxisListType.X, Alu.max,
                apply_absolute_value=True,
            )
            s_t = sp.tile([P, 1], f32, tag="s")
            nc.vector.tensor_scalar(s_t[:], mx[:], float(np.float32(1.0) / np.float32(QMAX)), None, Alu.mult)
            r_t = sp.tile([P, 1], f32, tag="r")
            nc.vector.reciprocal(r_t[:], s_t[:])

            # outlier activations: gather + pre-scale by r, transpose via PE
            ao = aop.tile([P, fp_dim], f32, tag="ao")
            if "gather" in DBG:
                nc.vector.tensor_copy(ao[:], x_t[:, :fp_dim])
            else:
                nc.gpsimd.ap_gather(
                    ao[:, :, None],
                    x_t[:, :, None],
                    idxs[:],
                    channels=P,
                    num_elems=in_dim,
                    d=1,
                    num_idxs=fp_dim,
                )
            aos = aop.tile([P, fp_dim], f32, tag="aos")
            nc.vector.tensor_scalar(aos[:], ao[:], r_t[:], None, Alu.mult)
            aoT = aotp.tile([P, FT, P], f32r)
            for ff in range(FT):
                ps = ptp.tile([P, P], f32, tag="tp")
                nc.tensor.transpose(ps[:], aos[:, ff * P : (ff + 1) * P], identity[:])
                nc.scalar.activation(aoT[:, ff, :], ps[:], Act.Copy)

            # quantize: t = xz * r + MAGIC (ScalarE), q = t - MAGIC (GPSIMD)
            nc.scalar.activation(
                x_t[:], xz[:], Act.Copy, bias=MAGIC, scale=r_t[:]
            )
            q = qp.tile([P, in_dim], bf16)
            if "gpsimdq" in DBG:
                nc.vector.tensor_scalar(q[:], x_t[:], -MAGIC, None, Alu.add)
            else:
                nc.gpsimd.tensor_scalar(q[:], x_t[:], -MAGIC, None, Alu.add)

            # transpose q to contraction-major via DMA xbar
            qT = qtp.tile([P, KT, P], bf16)
            nc.sync.dma_start_transpose(qT[:], q[:])

            # GEMMs: 32 int tiles + 2 outlier tiles accumulate in one bank
            psum = py.tile([P, os_dim], f32)
            for ko in range(KT):
                nc.tensor.matmul(
                    psum[:],
                    qT[:, ko, :],
                    wT[:, ko, :],
                    start=(ko == 0),
                    stop=False,
                )
            for ff in range(FT):
                nc.tensor.matmul(
                    psum[:],
                    aoT[:, ff, :],
                    wcT[:, ff, :],
                    start=False,
                    stop=(ff == FT - 1),
                )

            # dequant + store: y = psum * x_scale (ACT) * scale_col (DVE)
            t1 = yp.tile([P, os_dim], f32, tag="t1")
            nc.scalar.activation(t1[:], psum[:], Act.Copy, scale=s_t[:])
            yt = yp.tile([P, os_dim], f32, tag="yt")
            nc.vector.scalar_tensor_tensor(
                yt[:], t1[:], 1.0, sc_bcast[:], Alu.mult, Alu.mult
            )
            nc.sync.dma_start(y[mi * P : (mi + 1) * P, :], yt[:])

    return nc


def build_nc(m=M, in_dim=IN, os_dim=OS, fp_dim=FP):
    import concourse.bacc as bacc
    import concourse.tile as tile

    nc = bacc.Bacc(None, target_bir_lowering=False)
    with tile.TileContext(nc) as tc:
        emit_core_kernel(nc, tc, m, in_dim, os_dim, fp_dim)
    nc.compile()
    return nc


def make_host_inputs(x, q_weight, scale_col, weight_cache, ind,
                     m=M, in_dim=IN, os_dim=OS, fp_dim=FP, ncores=NCORES):
    """Shard/relayout full inputs into per-core input maps (no arithmetic)."""
    xf = np.ascontiguousarray(x.reshape(m, in_dim).astype(np.float32, copy=False))
    ind = np.asarray(ind).astype(np.int64)
    maskrow = np.ones(in_dim, dtype=np.float32)
    maskrow[ind] = 0.0
    w = ind.astype(np.int16).reshape(fp_dim // 16, 16)  # j = i*16 + (p%16)
    idx = np.tile(w.T, (8, 1)).astype(np.int16)  # [128, fp/16]
    scf = np.asarray(scale_col).reshape(-1).astype(np.float32, copy=False)

    in_maps = []
    for c in range(ncores):
        o0, o1 = c * os_dim, (c + 1) * os_dim
        in_maps.append(
            {
                "x": xf,
                "qw": np.ascontiguousarray(q_weight[o0:o1]).astype(np.int32, copy=False),
                "sc": np.ascontiguousarray(scf[o0:o1]),
                "wc": np.ascontiguousarray(weight_cache[o0:o1]).astype(np.float32, copy=False),
                "maskrow": maskrow,
                "idx": idx,
            }
        )
    return in_maps


_NC_CACHE = {}


def kernel(x, q_weight, scale_col, weight_cache, ind, trace=False):
    from concourse.bass_utils import run_bass_kernel_spmd

    key = "full"
    if key not in _NC_CACHE:
        _NC_CACHE[key] = build_nc()
    nc = _NC_CACHE[key]

    in_maps = make_host_inputs(x, q_weight, scale_col, weight_cache, ind)
    res = run_bass_kernel_spmd(nc, in_maps, list(range(NCORES)), trace=trace)
    yshards = [res.results[c]["y"] for c in range(NCORES)]
    yfull = np.concatenate(yshards, axis=1).reshape(B, S, OUT)
    if trace:
        return yfull, res
    return yfull



# revision 2
# speedup vs baseline: 1.0813x; 1.0813x over previous
"""MixLinear int4-GEMM kernel for 8x TRN2 NeuronCores.

Sharding: 4-way over rows (M) x 2-way over out_features; each core gets
rows shard [2048, 4096] and a 2048-wide out-feature shard. The main loop
is software-pipelined (load | quantize | matmul+store stages with lag) so
the per-tile chain overlaps across engines; x loads are spread over the
three DMA rings (Act/Pool/SP hwdge+swdge dispatch queues).

Per core, per 128-row tile:
  1. GPSIMD ap_gather pulls the 256 outlier activations; DVE zeroes the
     outlier columns in place (x * mask, bf16 mask) and an abs-max reduce
     gives the exact masked row scale s = max|x_kept|/7, r = 1/s.
  2. ScalarE quantizes with the magic-number RNE trick (two activation
     passes: t = x*r + 1.5*2^23 in f32, q = t - magic in bf16), with
     rowsum(q) accumulated for free via accum_out.
  3. q | aos (outlier activations * r) are packed in one [128, 4352] bf16
     tile, transposed by a single DMA-xbar, and the 32 int k-tiles
     converted to fp8e4.
  4. Int GEMM runs in fp8 DoubleRow mode (2 k-tiles per instruction, ~2x
     bf16 PE rate). Weights are stored as w' = nibble^8 = w+8 in [0,15]
     (exact in fp8); the -8 offset is folded into the dequant bias
     -8*rowsum(q)*s. Outlier GEMM stays bf16 against weight_cache/sc.
  5. Dequant: ScalarE psum evict (psum*s + bias -> bf16), DVE multiply by
     scale_col (bf16), DMA out; the host upcasts bf16 -> f32.
"""

import os as _os

import numpy as np

B, S, IN, OUT, FP = 4, 2048, 4096, 4096, 256
M = B * S
NCORES = 8
MSPLIT, OSPLIT = (
    (2, 4) if _os.environ.get("KERNEL_SHARD", "4x2") == "2x4" else (4, 2)
)
MSH = M // MSPLIT    # rows per core
OS = OUT // OSPLIT   # out-features per core
QMAX = 7.0
MAGIC = 12582912.0   # 1.5 * 2**23
P = 128
KT = IN // P         # 32 int k-tiles
NKT = KT + 2         # + 2 outlier (aos) k-tiles
KEPT = IN - FP       # 3840


def emit_core_kernel(nc, tc, ne_even):
    import concourse.bass as bass  # noqa: F401
    import concourse.mybir as mybir

    f32 = mybir.dt.float32
    bf16 = mybir.dt.bfloat16
    f8 = mybir.dt.float8e4
    i32 = mybir.dt.int32
    i16 = mybir.dt.int16
    Alu = mybir.AluOpType
    Act = mybir.ActivationFunctionType
    DR = mybir.MatmulPerfMode.DoubleRow

    MT = MSH // P            # 16 row tiles
    OJ = OS // P             # 16 out-feature chunks of 128
    MMW = int(_os.environ.get("KERNEL_MMW", "512"))  # matmul rhs width
    NB = OS // MMW           # matmul column blocks
    BF16 = _os.environ.get("KERNEL_BF16", "0") == "1"  # disable fp8 path
    CONV = _os.environ.get("KERNEL_CONV", "act")  # qT fp8 convert engine
    wdt = bf16 if BF16 else f8

    x = nc.dram_tensor("x", [MSH, IN], f32, kind="ExternalInput")
    qw = nc.dram_tensor("qw", [OS, IN // 2], mybir.dt.uint8, kind="ExternalInput")
    sc = nc.dram_tensor("sc", [OS], f32, kind="ExternalInput")
    wc = nc.dram_tensor("wc", [OS, FP], f32, kind="ExternalInput")
    idx_ao = nc.dram_tensor("idx_ao", [P, FP // 16], i16, kind="ExternalInput")
    maskrow = nc.dram_tensor("maskrow", [IN], bf16, kind="ExternalInput")
    y = nc.dram_tensor("y", [MSH, OS], bf16, kind="ExternalOutput")

    from concourse import library_config

    nc.gpsimd.load_library(library_config.ap_gather)

    with tc.tile_pool(name="persist", bufs=1) as pers:
        # persistent tiles
        wT8 = pers.tile([P, KT, OS], wdt)            # int weights, k-major
        wcT = pers.tile([P, 2 * OJ, P], bf16)        # outlier fp weights (interleaved)
        sc_bf = pers.tile([P, OS], bf16)             # scale_col broadcast
        ia = pers.tile([P, FP // 16], i16)
        nc.sync.dma_start(ia[:], idx_ao[:])
        maskF = pers.tile([P, IN], bf16)
        nc.sync.dma_start(maskF[:], maskrow[None, :].to_broadcast((P, IN)))

        # ------------------------- setup phase -------------------------
        with (
            tc.tile_pool(name="wstage", bufs=2) as ws,
            tc.tile_pool(name="wstage1", bufs=1) as ws1,
        ):
            # scale_col broadcast -> bf16
            scb32 = ws1.tile([P, OS], f32)
            nc.sync.dma_start(scb32[:], sc[None, :].to_broadcast((P, OS)))
            nc.vector.tensor_copy(sc_bf[:], scb32[:])

            # per-partition scale_col view + reciprocal (for wc prescale)
            scp = ws1.tile([P, OJ], f32)
            nc.sync.dma_start(scp[:], sc.rearrange("(j p) -> p j", p=P))
            rscp = ws1.tile([P, OJ], f32)
            nc.vector.reciprocal(rscp[:], scp[:])

            # weight_cache: wcp[o, f] = wc[o, f] / sc[o]  (ScalarE, bf16 out)
            wc_sb = ws1.tile([P, OJ, FP], f32)
            nc.sync.dma_start(wc_sb[:], wc.rearrange("(j p) f -> p j f", p=P))
            wcp = ws1.tile([P, OJ, FP], bf16)
            for j in range(OJ):
                nc.scalar.activation(
                    wcp[:, j, :], wc_sb[:, j, :], Act.Copy, scale=rscp[:, j : j + 1]
                )
            # one xbar: [128, OJ*FP] -> [128, OJ*FP/128, 128]; tile t=2j+fh
            nc.sync.dma_start_transpose(wcT[:], wcp[:])

            # int4 weights + gathered outlier rows, per 128-out-channel chunk
            qw_v = qw.rearrange("(j p) k -> p j k", p=P)
            for j in range(OJ):
                qwj = ws.tile([P, IN // 2], i32, tag="qwj")
                # u8 -> i32 widening cast happens inside the (gpsimd) DMA
                nc.gpsimd.dma_start(qwj[:], qw_v[:, j, :])
                wtmp = ws.tile([P, IN // 2, 2], i32, tag="wtmp")
                # low nibble -> even cols: (v & 15) ^ 8  (= w + 8 in [0,15])
                nc.vector.tensor_scalar(
                    wtmp[:, :, 0], qwj[:], 15, 8, Alu.bitwise_and, Alu.bitwise_xor
                )
                # high nibble -> odd cols: (v >> 4) ^ 8
                nc.vector.tensor_scalar(
                    wtmp[:, :, 1], qwj[:], 4, 8, Alu.arith_shift_right, Alu.bitwise_xor
                )
                w_ok = ws.tile([P, IN // 2, 2], bf16, tag="wok")
                nc.vector.tensor_copy(w_ok[:], wtmp[:])
                # transpose to k-major and convert to fp8
                wtb = ws.tile([P, KT, P], bf16, tag="wtb")
                nc.sync.dma_start_transpose(wtb[:], w_ok[:])
                nc.scalar.activation(
                    wT8[:, :, j * P : (j + 1) * P], wtb[:], Act.Copy
                )


        # ------------------------- main loop -------------------------
        with (
            tc.tile_pool(name="xp", bufs=4) as xp,
            tc.tile_pool(name="qp", bufs=1) as qp,
            tc.tile_pool(name="qtp", bufs=2) as qtp,
            tc.tile_pool(name="qt8p", bufs=2) as qt8p,
            tc.tile_pool(name="aop", bufs=2) as aop,
            tc.tile_pool(name="sp", bufs=4) as sp,
            tc.tile_pool(name="yp", bufs=2) as yp,
            tc.tile_pool(name="pp", bufs=2, space="PSUM") as pp,
        ):
            state = {}

            def emit_load(mi):
                x_t = xp.tile([P, IN], f32)
                rows = slice(mi * P, (mi + 1) * P)
                # x spread over all three DMA rings
                nc.scalar.dma_start(x_t[:, :1408], x[rows, :1408])
                nc.gpsimd.dma_start(x_t[:, 1408:2816], x[rows, 1408:2816])
                nc.sync.dma_start(x_t[:, 2816:], x[rows, 2816:])
                state[("x", mi)] = x_t

            def emit_quant(mi):
                x_t = state.pop(("x", mi))
                # outlier activations first (gather cost ~25ns/idx)
                aot = aop.tile([P, FP], f32, tag="ao")
                nc.gpsimd.ap_gather(
                    aot[:, :, None], x_t[:, :, None], ia[:],
                    channels=P, num_elems=IN, d=1, num_idxs=FP,
                )
                ao = aot[:]
                # zero outlier columns in place, then plain abs-max reduce
                nc.vector.tensor_tensor(x_t[:], x_t[:], maskF[:], Alu.mult)
                mx = sp.tile([P, 1], f32, tag="mx")
                nc.vector.tensor_reduce(
                    mx[:], x_t[:], mybir.AxisListType.X, Alu.max,
                    apply_absolute_value=True,
                )
                s_t = sp.tile([P, 1], f32, tag="s")
                nc.vector.tensor_scalar(
                    s_t[:], mx[:], float(np.float32(1.0) / np.float32(QMAX)),
                    None, Alu.mult,
                )
                r_t = sp.tile([P, 1], f32, tag="r")
                nc.vector.reciprocal(r_t[:], s_t[:])

                # quantize: t = x*r + MAGIC (f32), q = t - MAGIC (bf16 + rowsum)
                nc.scalar.activation(
                    x_t[:], x_t[:], Act.Copy, bias=MAGIC, scale=r_t[:]
                )
                qa = qp.tile([P, NKT * P], bf16)
                rsq = sp.tile([P, 1], f32, tag="rsq")
                nc.scalar.activation(
                    qa[:, :IN], x_t[:], Act.Copy, bias=-MAGIC, accum_out=rsq[:]
                )
                # outlier activations: aos = ao * r (bf16)
                nc.scalar.activation(
                    qa[:, IN :], ao, Act.Copy, scale=r_t[:]
                )

                # dequant bias: -8 * rsq * s
                s8 = sp.tile([P, 1], f32, tag="s8")
                nc.vector.tensor_scalar(s8[:], s_t[:], -8.0, None, Alu.mult)
                bias_t = sp.tile([P, 1], f32, tag="bias")
                nc.vector.tensor_tensor(bias_t[:], rsq[:], s8[:], Alu.mult)

                # transpose all operands in one xbar; convert first 34 to fp8
                qT = qtp.tile([P, NKT, P], bf16)
                nc.sync.dma_start_transpose(qT[:], qa[:])
                if BF16:
                    qT8 = qT
                else:
                    qT8 = qt8p.tile([P, KT, P], f8)
                    if CONV == "act":
                        nc.scalar.activation(qT8[:], qT[:, :KT, :], Act.Copy)
                    elif CONV == "split":
                        nc.vector.tensor_copy(
                            qT8[:, : KT // 2, :], qT[:, : KT // 2, :]
                        )
                        nc.scalar.activation(
                            qT8[:, KT // 2 :, :], qT[:, KT // 2 : KT, :], Act.Copy,
                        )
                    else:
                        nc.vector.tensor_copy(qT8[:], qT[:, :KT, :])
                state[("q", mi)] = (qT, qT8, s_t, bias_t)

            def emit_mm(mi):
                qT, qT8, s_t, bias_t = state.pop(("q", mi))
                # GEMMs: 16 fp8-DR int pairs + 1 DR correction + 2 bf16 outlier
                psum = pp.tile([P, OS], f32)
                JB = MMW // P  # out-feature chunks per column block
                for b in range(NB):
                    cs = slice(b * MMW, (b + 1) * MMW)
                    if BF16:
                        for ko in range(KT):
                            nc.tensor.matmul(
                                psum[:, cs], qT8[:, ko, :], wT8[:, ko, cs],
                                start=(ko == 0), stop=False,
                            )
                    else:
                        for kp in range(KT // 2):
                            nc.tensor.matmul(
                                psum[:, cs],
                                qT8[:, 2 * kp : 2 * kp + 2, :],
                                wT8[:, 2 * kp : 2 * kp + 2, cs],
                                start=(kp == 0), stop=False, perf_mode=DR,
                            )
                    for fh in range(2):
                        nc.tensor.matmul(
                            psum[:, cs],
                            qT[:, KT + fh, :],
                            wcT[:, 2 * JB * b + fh : 2 * JB * b + fh + 2 * JB - 1 : 2, :],
                            start=False, stop=(fh == 1),
                        )

                # dequant + store (sc multiply in-place to save SBUF)
                t1 = yp.tile([P, OS], bf16, tag="t1")
                nc.scalar.activation(
                    t1[:], psum[:], Act.Identity, scale=s_t[:], bias=bias_t[:]
                )
                nc.vector.tensor_tensor(t1[:], t1[:], sc_bf[:], Alu.mult)
                nc.sync.dma_start(y[mi * P : (mi + 1) * P, :], t1[:])

            # software pipeline: load(t) | quant(t-2) | matmul+store(t-3)
            for t in range(MT + 3):
                if t < MT:
                    emit_load(t)
                if 0 <= t - 2 < MT:
                    emit_quant(t - 2)
                if 0 <= t - 3 < MT:
                    emit_mm(t - 3)

    return nc


def build_nc(ne_even):
    import concourse.bacc as bacc
    import concourse.tile as tile

    nc = bacc.Bacc(None, target_bir_lowering=False)
    with tile.TileContext(nc) as tc:
        emit_core_kernel(nc, tc, ne_even)
    nc.compile()
    return nc


def wrap_idx(v):
    w = np.asarray(v).astype(np.int16).reshape(-1, 16)
    return np.ascontiguousarray(np.tile(w.T, (8, 1)).astype(np.int16))


def make_host_inputs(x, q_weight, scale_col, weight_cache, ind):
    """Shard/relayout full inputs into per-core input maps (no arithmetic)."""
    xf = np.ascontiguousarray(x.reshape(M, IN).astype(np.float32, copy=False))
    ind = np.asarray(ind).astype(np.int64)
    evens = ind[ind % 2 == 0]
    odds = ind[ind % 2 == 1]
    ind_perm = np.concatenate([evens, odds])
    perm = np.concatenate([np.where(ind % 2 == 0)[0], np.where(ind % 2 == 1)[0]])
    kept = np.setdiff1d(np.arange(IN), ind)
    scf = np.asarray(scale_col).reshape(-1).astype(np.float32, copy=False)
    wcf = np.asarray(weight_cache).astype(np.float32, copy=False)[:, perm]

    import ml_dtypes
    idx_ao = wrap_idx(ind_perm)
    maskrow = np.ones(IN, dtype=ml_dtypes.bfloat16)
    maskrow[ind] = 0

    in_maps = []
    for c in range(NCORES):
        cm, co = c // OSPLIT, c % OSPLIT
        m0, o0 = cm * MSH, co * OS
        in_maps.append(
            {
                "x": xf[m0 : m0 + MSH],
                "qw": np.ascontiguousarray(q_weight[o0 : o0 + OS]).astype(
                    np.uint8
                ),
                "sc": np.ascontiguousarray(scf[o0 : o0 + OS]),
                "wc": np.ascontiguousarray(wcf[o0 : o0 + OS]),
                "idx_ao": idx_ao,
                "maskrow": maskrow,
            }
        )
    return in_maps, len(evens)


_NC_CACHE = {}


def kernel(x, q_weight, scale_col, weight_cache, ind, trace=False):
    from concourse.bass_utils import run_bass_kernel_spmd

    in_maps, ne_even = make_host_inputs(x, q_weight, scale_col, weight_cache, ind)
    key = ne_even
    if key not in _NC_CACHE:
        _NC_CACHE[key] = build_nc(ne_even)
    nc = _NC_CACHE[key]

    res = run_bass_kernel_spmd(nc, in_maps, list(range(NCORES)), trace=trace)
    yshards = [
        np.asarray(res.results[c]["y"]).astype(np.float32) for c in range(NCORES)
    ]
    rows = [
        np.concatenate(yshards[cm * OSPLIT : (cm + 1) * OSPLIT], axis=1)
        for cm in range(MSPLIT)
    ]
    yfull = np.concatenate(rows, axis=0).reshape(B, S, OUT)
    if trace:
        return yfull, res
    return yfull
